# revision 1
# baseline (speedup 1.0000x reference)
"""Self-contained Trainium2 Bass kernel for nn_AutoRegressive_88837103551116.

2-layer LSTM (HID=64) over ragged sequences: warmup pass over x (per-sample
lengths), then autoregressive decode over [dense(h_top_final), context_t].
Pure data-parallel over 8 NeuronCores (batch 512 -> 64 per core).

Device algorithm (per core):
  - the 64-sample batch is split into TWO independent 32-sample chains that
    interleave on the engines: each chain's recurrence latency hides the
    other's, doubling timestep throughput
  - slot s computes layer0 @ step s and layer1 @ step s-1 (layer stagger)
  - states [feature, batch]: rb [128,Bc]=[h0;h1] (fp16); tcc [128,2Bc] with
    partitions 0:64 = tanh(g) scratch and 64:128 = c state (fp32), so
    i*tanh(g) and f*c merge into one DVE multiply
  - all matmuls fp16 (1 cycle/row on PE); gates accumulate in fp32 PSUM;
    per chain+parity ONE PSUM bank [128,3,64] = [IF | GO | junk]: the junk
    block overlaps both leading matmuls' writes, forcing their order so the
    bank's accumulation group is opened exactly once
  - ONE unified 19-row input stream per slot carries x/ctx (rows 0:16), the
    ragged-sequence c-freeze mask row (+/-BIG into i/f preactivations past
    each sample's length) and per-layer bias rows for both layer column
    halves -> a single leading matmul per gate block; only the two
    h-dependent matmuls per block sit on the recurrence critical path
  - decode folds the constant warmup element into stream rows 8:16 (device
    broadcasts elem into the stream tiles once, DMA skips those rows)
  - h at the last valid warmup step is captured into hkeep via
    copy_predicated with uint8 mask streams (off the critical path)
  - host side: input transposes/stream building, output -999 masking
"""
import sys

import numpy as np

try:
    import concourse.bass as bass
except ImportError:
    sys.path.insert(0, "/opt/trn_rl_repo")
    import concourse.bass as bass

import contextlib
import json

import concourse.tile as tile
from concourse import mybir
from concourse.bass_utils import run_bass_kernel_spmd

N_CORES = 8
TW = 512
TC = 512

H = 64
IN = 16
F = 8
C = 8
BIG = 50.0

F32 = mybir.dt.float32
F16 = mybir.dt.float16
MMDT = F32          # dtype for matmul weights/streams/h-state
NPDT = "float32"   # matching numpy dtype for host-side builders
U8 = mybir.dt.uint8
AF = mybir.ActivationFunctionType

B = 64     # batch per core
Bc = 32    # batch per chain
SW = 528   # warmup slots (padded; needs >= 513)
SD = 512   # decode slots (l0 steps 0..510 + 1 pad)
CW = SW // 8
CD = SD // 8


def build_weights(Wih0, Whh0, bih0, bhh0, Wih1, Whh1, bih1, bhh1, Wd, bd):
    """Stationary lhsT matrices (shared across cores), fp16."""
    b0 = bih0 + bhh0
    b1 = bih1 + bhh1
    maskcol_if = np.concatenate([np.full(H, -BIG), np.full(H, BIG)]).astype(np.float32)
    zero = np.zeros(128, np.float32)

    def stack19(xw_rows, gate_rows, mask):
        # lhsT [19, 128]: rows 0:16 x-weights, 16 mask, 17 b0, 18 b1
        out = np.zeros((19, 128), np.float32)
        out[0:xw_rows.shape[0], :] = xw_rows
        out[16] = mask
        out[17] = b0[gate_rows]
        out[18] = b1[gate_rows]
        return out

    gi = slice(0, 128)    # i,f rows
    gg = slice(128, 256)  # g,o rows
    W = {}
    W["wx_if"] = stack19(Wih0[gi].T, gi, maskcol_if)
    W["wx_go"] = stack19(Wih0[gg].T, gg, zero)
    # decode l0 stream rows: 0:8 = elem (cols 0:8 of Wih0, device-written at
    # partition base 0), 8:16 = ctx (cols 8:16)
    dxi = Wih0[gi].T
    dxg = Wih0[gg].T
    W["dx_if"] = stack19(dxi, gi, maskcol_if)
    W["dx_go"] = stack19(dxg, gg, zero)
    W["w0h_if"] = Whh0[gi].T.copy()   # [64,128]
    W["w0h_go"] = Whh0[gg].T.copy()
    W["w1_if"] = np.concatenate([Wih1[gi].T, Whh1[gi].T], 0)  # [128,128]
    W["w1_go"] = np.concatenate([Wih1[gg].T, Whh1[gg].T], 0)
    W["wdT"] = Wd.T.copy()  # [64,8]
    for k in W:
        W[k] = np.ascontiguousarray(W[k], NPDT)
    W["bd"] = np.ascontiguousarray(bd.reshape(8, 1), np.float32)
    return W


def build_streams(x, lengths_x, context, lengths_ctx):
    """Per-core streams. x [B,TW,16], context [B,TC,8].
    Slot columns: [c0: l0(32) l1(32) | c1: l0(32) l1(32)] where chain c =
    samples [c*32:(c+1)*32] of this core's batch."""
    Bn = x.shape[0]
    TWl = x.shape[1]
    TCl = context.shape[1]

    s_idx = np.arange(SW)
    mw = (s_idx[:, None] < lengths_x[None, :]).astype(np.float32)  # [SW,Bn]
    mw1 = np.zeros_like(mw)
    mw1[1:] = mw[:-1]

    # [CW, 19, 8, chain(2), layer(2), Bc]
    WA = np.zeros((CW, 19, 8, 2, 2, Bc), np.float32)
    xt = np.transpose(x, (1, 2, 0))  # [TW,16,Bn]
    xp = np.concatenate([xt, np.zeros((SW - TWl, 16, Bn), np.float32)], 0)
    WA[:, 0:16, :, :, 0, :] = xp.reshape(CW, 8, 16, 2, Bc).transpose(0, 2, 1, 3, 4)
    WA[:, 16, :, :, 0, :] = (1.0 - mw).reshape(CW, 8, 2, Bc)
    WA[:, 16, :, :, 1, :] = (1.0 - mw1).reshape(CW, 8, 2, Bc)
    WA[:, 17, :, :, 0, :] = 1.0
    WA[:, 18, :, :, 1, :] = 1.0

    # h-capture masks [CW, 128, 8, Bn]: rows 0:64 = mw (h0), 64:128 = mw1 (h1)
    NMw = np.zeros((CW, 128, 8, Bn), np.uint8)
    NMw[:, 0:64] = mw.reshape(CW, 8, 1, Bn).transpose(0, 2, 1, 3)
    NMw[:, 64:128] = mw1.reshape(CW, 8, 1, Bn).transpose(0, 2, 1, 3)

    md0 = np.zeros((SD, Bn), np.float32)
    md0[0:TCl - 1] = 1.0     # l0 steps 0..510 active; 511 pad frozen
    md1 = np.ones((SD, Bn), np.float32)
    md1[0] = 0.0             # freeze l1 at slot 0
    DA = np.zeros((CD, 19, 8, 2, 2, Bc), np.float32)
    ctxt = np.transpose(context, (1, 2, 0))  # [TC,8,Bn]
    cp = np.concatenate(
        [ctxt[0:TCl - 1], np.zeros((SD - (TCl - 1), 8, Bn), np.float32)], 0
    )
    DA[:, 8:16, :, :, 0, :] = cp.reshape(CD, 8, 8, 2, Bc).transpose(0, 2, 1, 3, 4)
    # rows 0:8 (elem) stay 0 in HBM; device fills SBUF copies
    DA[:, 16, :, :, 0, :] = (1.0 - md0).reshape(CD, 8, 2, Bc)
    DA[:, 16, :, :, 1, :] = (1.0 - md1).reshape(CD, 8, 2, Bc)
    DA[:, 17, :, :, 0, :] = 1.0
    DA[:, 18, :, :, 1, :] = 1.0

    def pad1(a):
        return np.concatenate([a, np.zeros_like(a[:1])], 0)
    return dict(
        wa=pad1(WA.reshape(CW, 19, 1024)).astype(NPDT),
        nmw=pad1(NMw.reshape(CW, 128, 512)),
        da=pad1(DA.reshape(CD, 19, 1024)).astype(NPDT),
    )


def post_outputs(YE, YD, bd, lengths_ctx, TCl):
    """YE [8,Bn] fp16, YD [CD,8,8,Bn] f32 -> out [Bn,TCl,8] with -999 pad.
    Chain split uses contiguous sample halves so no reordering is needed."""
    Bn = YE.shape[1]
    out = np.zeros((Bn, TCl, F), np.float32)
    out[:, 0, :] = YE.T.astype(np.float32)
    ysd = YD.transpose(0, 2, 1, 3).reshape(SD, F, Bn)  # [slot, F, Bn]
    # ys_t = slot t+1 for t = 0..510
    out[:, 1:TCl, :] = ysd[1:TCl].transpose(2, 0, 1) + bd[None, None, :]
    valid = np.arange(TCl)[None, :] < lengths_ctx[:, None]
    return np.where(valid[:, :, None], out, np.float32(-999.0))


def build_nc(repeat=1, static=False):
    nc = bass.Bass("TRN2", target_bir_lowering=False, debug=False)

    d = {}
    d["wa"] = nc.dram_tensor("wa", [CW + 1, 19, 1024], MMDT, kind="ExternalInput")
    d["nmw"] = nc.dram_tensor("nmw", [CW + 1, 128, 512], U8, kind="ExternalInput")
    d["da"] = nc.dram_tensor("da", [CD + 1, 19, 1024], MMDT, kind="ExternalInput")
    for name, shp, dt_ in [
        ("wx_if", [19, 128], MMDT), ("wx_go", [19, 128], MMDT),
        ("dx_if", [19, 128], MMDT), ("dx_go", [19, 128], MMDT),
        ("w0h_if", [64, 128], MMDT), ("w0h_go", [64, 128], MMDT),
        ("w1_if", [128, 128], MMDT), ("w1_go", [128, 128], MMDT),
        ("wdT", [64, 8], MMDT), ("bd", [8, 1], F32),
    ]:
        d[name] = nc.dram_tensor(name, shp, dt_, kind="ExternalInput")
    ye = nc.dram_tensor("ye", [8, B], MMDT, kind="ExternalOutput")
    yd = nc.dram_tensor("yd", [CD, 8, 512], F32, kind="ExternalOutput")

    with tile.TileContext(nc) as tc:
        with (
            tc.tile_pool(name="consts", bufs=1) as consts,
            tc.tile_pool(name="state", bufs=1) as state,
            tc.tile_pool(name="stream", bufs=1) as stream,
            tc.tile_pool(name="work", bufs=3) as work,
            tc.tile_pool(name="psum", bufs=2, space="PSUM") as psum,
            tc.tile_pool(name="outp", bufs=1, space="PSUM") as outp,
        ):
            W = {}
            for name in ["wx_if", "wx_go", "dx_if", "dx_go", "w0h_if",
                         "w0h_go", "w1_if", "w1_go"]:
                t = consts.tile(list(d[name].shape), MMDT, tag=name, name="w_" + name)
                nc.sync.dma_start(out=t, in_=d[name][:, :])
                W[name] = t
            wdT_t = consts.tile([128, 8], MMDT, tag="wdT", name="w_wdT")
            nc.sync.dma_start(out=wdT_t[64:128, :], in_=d["wdT"][:, :])
            W["wdT"] = wdT_t
            bd_t = consts.tile([8, 1], F32, tag="bd", name="w_bd")
            nc.sync.dma_start(out=bd_t, in_=d["bd"][:, :])
            W["bd"] = bd_t

            rb = [[state.tile([128, Bc], MMDT, tag=f"rb{c}{i}", name=f"rb{c}{i}")
                   for i in range(2)] for c in range(2)]
            tcc = [[state.tile([128, 2 * Bc], F32, tag=f"tcc{c}{i}", name=f"tcc{c}{i}")
                    for i in range(2)] for c in range(2)]
            for c in range(2):
                for i in range(2):
                    nc.vector.memset(rb[c][i], 0.0)
                    nc.vector.memset(tcc[c][i], 0.0)

            # stream tiles: 8 slots x 128 cols + 128 junk cols (lead matmuls
            # of chain 1 read 64 cols past their slot)
            saA = stream.tile([19, 1152], MMDT, tag="saA")
            saB = stream.tile([19, 1152], MMDT, tag="saB")
            nc.vector.memset(saA[:, 1024:1152], 0.0)
            nc.vector.memset(saB[:, 1024:1152], 0.0)
            nmA = stream.tile([128, 512], U8, tag="nmA")
            nmB = stream.tile([128, 512], U8, tag="nmB")
            elem = state.tile([8, B], MMDT, tag="elem")
            hkeep = state.tile([128, B], MMDT, tag="hkeep")
            nc.vector.memset(hkeep, 0.0)

            def tick_chain(sl, ch, sa, nm, decode, outmm=None):
                par = sl % 2
                rbp, rbn = rb[ch][par], rb[ch][1 - par]
                tccp, tccn = tcc[ch][par], tcc[ch][1 - par]
                t8 = sl % 8
                base = t8 * 128 + ch * 64
                # [IF(64) | GO(64) | junk(64)]; both leads write the junk
                # block so their order (group opener first) is preserved
                mega = psum.tile([128, 3, 64], F32, tag=f"mega{ch}", name=f"mega{ch}")

                wx_if = W["dx_if"] if decode else W["wx_if"]
                wx_go = W["dx_go"] if decode else W["wx_go"]

                nc.tensor.matmul(mega[:, 0::2, :], wx_if, sa[0:19, base:base + 128],
                                 start=True, stop=False)
                nc.tensor.matmul(mega[:, 1:3, :], wx_go, sa[0:19, base:base + 128],
                                 start=False, stop=False)
                nc.tensor.matmul(mega[:, 0, 0:Bc], W["w0h_if"], rbp[0:64, :],
                                 start=False, stop=False)
                nc.tensor.matmul(mega[:, 0, Bc:2 * Bc], W["w1_if"], rbp[:, :],
                                 start=False, stop=False)
                nc.tensor.matmul(mega[:, 1, 0:Bc], W["w0h_go"], rbp[0:64, :],
                                 start=False, stop=False)
                nc.tensor.matmul(mega[:, 1, Bc:2 * Bc], W["w1_go"], rbp[:, :],
                                 start=False, stop=True)
                if outmm is not None:
                    # dense(h1) of the PREVIOUS slot (rbp == rbn of slot-1):
                    # same data dependency as the gate matmuls above
                    ops, oc = outmm
                    nc.tensor.matmul(ops[:, oc * B + ch * Bc:oc * B + (ch + 1) * Bc],
                                     W["wdT"][64:128, :], rbp[64:128, :],
                                     start=True, stop=True)

                # sif = [sigmoid(i); sigmoid(f)] aligned with tcc = [tanh(g); c]
                # so every gate product has equal SBUF base partitions
                sif = work.tile([128, 2 * Bc], F32, tag=f"sif{ch}", name=f"sif{ch}")
                so = work.tile([64, 2 * Bc], F32, tag=f"so{ch}", name=f"so{ch}")
                nc.scalar.activation(sif, mega[:, 0, :], AF.Sigmoid)
                nc.scalar.activation(tccp[0:64, :], mega[0:64, 1, :], AF.Tanh)
                nc.scalar.activation(so, mega[64:128, 1, :], AF.Sigmoid)

                # c' = f*c + i*tanh(g); t2 first (needs only sif, not tg)
                t1 = work.tile([64, 2 * Bc], F32, tag=f"t1{ch}", name=f"t1{ch}")
                t2 = work.tile([64, 2 * Bc], F32, tag=f"t2{ch}", name=f"t2{ch}")
                th = work.tile([64, 2 * Bc], F32, tag=f"th{ch}", name=f"th{ch}")
                nc.vector.tensor_mul(t2, sif[64:128, :], tccp[64:128, :])
                nc.vector.tensor_mul(t1, sif[0:64, :], tccp[0:64, :])
                nc.vector.tensor_add(tccn[64:128, :], t1, t2)
                nc.scalar.activation(th, tccn[64:128, :], AF.Tanh)
                nc.vector.tensor_mul(rbn[0:64, :], so[:, 0:Bc], th[:, 0:Bc])
                nc.vector.tensor_mul(rbn[64:128, :], so[:, Bc:2 * Bc], th[:, Bc:2 * Bc])

                if nm is not None:
                    # capture h at each sample's last active slot
                    mc = t8 * 64 + ch * Bc
                    nc.vector.copy_predicated(hkeep[:, ch * Bc:(ch + 1) * Bc],
                                              nm[:, mc:mc + Bc], rbn)
                return rbn

            def tick(sl, sa, nm, decode, ops=None, oc=None):
                outmm = (ops, oc) if ops is not None else None
                r0 = tick_chain(sl, 0, sa, nm, decode, outmm)
                r1 = tick_chain(sl, 1, sa, nm, decode, outmm)
                return r0, r1

            rep_cm = tc.For_i(0, repeat, 1) if repeat > 1 else contextlib.nullcontext()
            with rep_cm:
                # ================= warmup =================
                nc.sync.dma_start(out=saA[:, 0:1024], in_=d["wa"][0, :, :])
                nc.sync.dma_start(out=nmA, in_=d["nmw"][0, :, :])
                def warm_body(j, i1, i2, first=False):
                    nc.sync.dma_start(out=saB[:, 0:1024], in_=d["wa"][i1, :, :])
                    nc.sync.dma_start(out=nmB, in_=d["nmw"][i1, :, :])
                    for sl in range(8):
                        tick(sl, saA, nmA, False)
                        if first and sl == 0:
                            for c in range(2):
                                nc.vector.memset(rb[c][1][64:128, :], 0.0)
                    nc.sync.dma_start(out=saA[:, 0:1024], in_=d["wa"][i2, :, :])
                    nc.sync.dma_start(out=nmA, in_=d["nmw"][i2, :, :])
                    for sl in range(8, 16):
                        tick(sl, saB, nmB, False)

                if static:
                    for j in range(CW // 2):
                        warm_body(j, j * 2 + 1, j * 2 + 2, first=(j == 0))
                else:
                    warm_body(0, 1, 2, first=True)
                    with tc.For_i(1, CW // 2, 1, hint_engines=(mybir.EngineType.PE,)) as j:
                        warm_body(j, nc.snap(j * 2 + 1), nc.snap(j * 2 + 2))

                # ================= elem =================
                pe = outp.tile([8, B], F32, tag="pe", name="pe")
                for c in range(2):
                    nc.vector.tensor_copy(rb[c][0], hkeep[:, c * Bc:(c + 1) * Bc])
                    nc.tensor.matmul(pe[:, c * Bc:(c + 1) * Bc], W["wdT"][64:128, :],
                                     rb[c][0][64:128, :], start=True, stop=True)
                nc.scalar.activation(elem, pe, AF.Identity, bias=W["bd"][:, 0:1])
                nc.sync.dma_start(out=ye[:, :], in_=elem)
                # broadcast elem into decode stream rows 0:8 (constant input)
                for buf in (saA, saB):
                    for k in range(8):
                        for c in range(2):
                            fb = k * 128 + c * 64
                            nc.vector.tensor_copy(buf[0:8, fb:fb + Bc],
                                                  elem[:, c * Bc:(c + 1) * Bc])
                            nc.vector.memset(buf[0:8, fb + Bc:fb + 64], 0.0)

                # ================= decode =================
                # DMA skips rows 0:8 so the device-written elem rows persist
                def dec_dma(buf, i):
                    nc.sync.dma_start(out=buf[8:19, 0:1024], in_=d["da"][i, 8:19, :])
                dec_dma(saA, 0)
                def dec_body(j, i0, i1, i2, first=False):
                    # tick k emits dense(h1) for slot k-1 (reading rbp); the
                    # last slot's output is emitted in the tail
                    dec_dma(saB, i1)
                    ops = outp.tile([8, 512], F32, tag="ops", name="ops")
                    for sl in range(8):
                        if sl >= 1:
                            tick(sl, saA, None, True, ops, sl - 1)
                        else:
                            tick(sl, saA, None, True)
                        if first and sl == 0:
                            for c in range(2):
                                nc.vector.tensor_copy(rb[c][1][64:128, :],
                                                      rb[c][0][64:128, :])
                    dec_dma(saA, i2)
                    ops2 = outp.tile([8, 512], F32, tag="ops2", name="ops2")
                    rs = None
                    for sl in range(8, 16):
                        if sl == 8:
                            rs = tick(sl, saB, None, True, ops, 7)
                            # PSUM can't be DMA'd directly; the ACT copy here
                            # lands in a queue gap behind this tick's acts
                            oso = work.tile([8, 512], F32, tag="oso", name="oso")
                            nc.scalar.copy(oso, ops)
                            nc.sync.dma_start(out=yd[i0, :, :], in_=oso)
                        else:
                            rs = tick(sl, saB, None, True, ops2, sl - 9)
                    for c in range(2):
                        nc.tensor.matmul(ops2[:, 7 * B + c * Bc:7 * B + (c + 1) * Bc],
                                         W["wdT"][64:128, :], rs[c][64:128, :],
                                         start=True, stop=True)
                    oso2 = work.tile([8, 512], F32, tag="oso2", name="oso2")
                    nc.scalar.copy(oso2, ops2)
                    nc.sync.dma_start(out=yd[i1, :, :], in_=oso2)

                if static:
                    for j in range(CD // 2):
                        dec_body(j, j * 2, j * 2 + 1, j * 2 + 2, first=(j == 0))
                else:
                    dec_body(0, 0, 1, 2, first=True)
                    with tc.For_i(1, CD // 2, 1, hint_engines=(mybir.EngineType.PE,)) as j:
                        dec_body(j, nc.snap(j * 2), nc.snap(j * 2 + 1), nc.snap(j * 2 + 2))

    return nc


def legalize_waits(nc, max_waits=1):
    """walrus codegen caps semaphore waits per instruction; move extras onto
    NoOp instructions inserted immediately before (same engine)."""
    j = json.loads(mybir.module_to_json_bytes(nc.m))
    for fn in j.get("functions", []):
        for blk in fn.get("blocks", []):
            out = []
            for inst in blk.get("instructions", []):
                si = inst.get("sync_info") or {}
                waits = si.get("on_wait") or []
                if len(waits) > max_waits:
                    keep, extra = waits[-max_waits:], waits[:-max_waits]
                    for k, w in enumerate(extra):
                        out.append({"name": f"{inst['name']}-wsp{k}",
                                    "opcode": "NoOp", "engine": inst["engine"],
                                    "ins": [], "outs": [],
                                    "sync_info": {"on_wait": [w], "on_update": []}})
                    si = dict(si); si["on_wait"] = keep
                    inst = dict(inst); inst["sync_info"] = si
                out.append(inst)
            blk["instructions"] = out
    nc.m = mybir.module_from_json_bytes(json.dumps(j).encode())
    return nc


_NC_CACHE = {}


def _get_nc(repeat=1):
    if repeat not in _NC_CACHE:
        nc = build_nc(repeat)
        legalize_waits(nc)
        _NC_CACHE[repeat] = nc
    return _NC_CACHE[repeat]


def build_in_maps(x, lengths_x, context, lengths_ctx,
                  Wih0, Whh0, bih0, bhh0, Wih1, Whh1, bih1, bhh1, Wd, bd):
    Wt = build_weights(np.asarray(Wih0, np.float32), np.asarray(Whh0, np.float32),
                       np.asarray(bih0, np.float32), np.asarray(bhh0, np.float32),
                       np.asarray(Wih1, np.float32), np.asarray(Whh1, np.float32),
                       np.asarray(bih1, np.float32), np.asarray(bhh1, np.float32),
                       np.asarray(Wd, np.float32), np.asarray(bd, np.float32))
    Bn = x.shape[0] // N_CORES
    in_maps = []
    for core in range(N_CORES):
        sl = slice(core * Bn, (core + 1) * Bn)
        st = build_streams(np.ascontiguousarray(x[sl], dtype=np.float32),
                           np.asarray(lengths_x[sl], dtype=np.int64),
                           np.ascontiguousarray(context[sl], dtype=np.float32),
                           np.asarray(lengths_ctx[sl], dtype=np.int64))
        m = dict(st)
        m.update(Wt)
        in_maps.append(m)
    return in_maps


def kernel(x, lengths_x, context, lengths_ctx,
           Wih0, Whh0, bih0, bhh0, Wih1, Whh1, bih1, bhh1, Wd, bd):
    x = np.asarray(x)
    context = np.asarray(context)
    lengths_x = np.asarray(lengths_x)
    lengths_ctx = np.asarray(lengths_ctx)
    in_maps = build_in_maps(x, lengths_x, context, lengths_ctx,
                            np.asarray(Wih0), np.asarray(Whh0), np.asarray(bih0),
                            np.asarray(bhh0), np.asarray(Wih1), np.asarray(Whh1),
                            np.asarray(bih1), np.asarray(bhh1), np.asarray(Wd),
                            np.asarray(bd))
    nc = _get_nc(1)
    res = run_bass_kernel_spmd(nc, in_maps, core_ids=list(range(N_CORES)))
    Bn = x.shape[0] // N_CORES
    outs = []
    bd32 = np.asarray(bd, dtype=np.float32)
    for core in range(N_CORES):
        sl = slice(core * Bn, (core + 1) * Bn)
        YE = res.results[core]["ye"]
        YD = res.results[core]["yd"].reshape(CD, 8, 8, Bn)
        outs.append(post_outputs(YE, YD, bd32,
                                 np.asarray(lengths_ctx[sl], dtype=np.int64), TC))
    return np.concatenate(outs, axis=0).astype(np.float32)



# revision 2
# speedup vs baseline: 2.3706x; 2.3706x over previous
"""Self-contained Trainium2 Bass kernel for nn_AutoRegressive_88837103551116.

2-layer LSTM (HID=64) over ragged sequences: warmup pass over x (per-sample
lengths), then autoregressive decode over [dense(h_top_final), context_t].
Pure data-parallel over 8 NeuronCores (batch 512 -> 64 per core).

Device algorithm (per core):
  - the 64-sample batch is split into TWO independent 32-sample chains that
    interleave on the engines: each chain's recurrence latency hides the
    other's, doubling timestep throughput
  - slot s computes layer0 @ step s and layer1 @ step s-1 (layer stagger)
  - states [feature, batch]: rb [128,Bc]=[h0;h1] (fp16); tcc [128,2Bc] with
    partitions 0:64 = tanh(g) scratch and 64:128 = c state (fp32), so
    i*tanh(g) and f*c merge into one DVE multiply
  - all matmuls fp16 (1 cycle/row on PE); gates accumulate in fp32 PSUM;
    per chain+parity ONE PSUM bank [128,3,64] = [IF | GO | junk]: the junk
    block overlaps both leading matmuls' writes, forcing their order so the
    bank's accumulation group is opened exactly once
  - ONE unified 19-row input stream per slot carries x/ctx (rows 0:16), the
    ragged-sequence c-freeze mask row (+/-BIG into i/f preactivations past
    each sample's length) and per-layer bias rows for both layer column
    halves -> a single leading matmul per gate block; only the two
    h-dependent matmuls per block sit on the recurrence critical path
  - decode folds the constant warmup element into stream rows 8:16 (device
    broadcasts elem into the stream tiles once, DMA skips those rows)
  - h at the last valid warmup step is captured into hkeep via
    copy_predicated with uint8 mask streams (off the critical path)
  - host side: input transposes/stream building, output -999 masking
"""
import sys

import numpy as np

try:
    import concourse.bass as bass
except ImportError:
    sys.path.insert(0, "/opt/trn_rl_repo")
    import concourse.bass as bass

import contextlib
import json

import concourse.tile as tile
from concourse import mybir
from concourse.bass_utils import run_bass_kernel_spmd

N_CORES = 8
TW = 512
TC = 512

H = 64
IN = 16
F = 8
C = 8
BIG = 50.0

F32 = mybir.dt.float32
F16 = mybir.dt.float16
MMDT = F16          # dtype for matmul weights/streams/h-state
NPDT = "float16"   # matching numpy dtype for host-side builders
U8 = mybir.dt.uint8
AF = mybir.ActivationFunctionType

B = 64     # batch per core
Bc = 32    # batch per chain
SW = 528   # warmup slots (padded; needs >= 513)
SD = 512   # decode slots (l0 steps 0..510 + 1 pad)
CW = SW // 8
CD = SD // 8


def build_weights(Wih0, Whh0, bih0, bhh0, Wih1, Whh1, bih1, bhh1, Wd, bd):
    """Stationary lhsT matrices (shared across cores), fp16."""
    b0 = bih0 + bhh0
    b1 = bih1 + bhh1
    maskcol_if = np.concatenate([np.full(H, -BIG), np.full(H, BIG)]).astype(np.float32)
    zero = np.zeros(128, np.float32)

    def stack19(xw_rows, gate_rows, mask):
        # lhsT [19, 128]: rows 0:16 x-weights, 16 mask, 17 b0, 18 b1
        out = np.zeros((19, 128), np.float32)
        out[0:xw_rows.shape[0], :] = xw_rows
        out[16] = mask
        out[17] = b0[gate_rows]
        out[18] = b1[gate_rows]
        return out

    gi = slice(0, 128)    # i,f rows
    gg = slice(128, 256)  # g,o rows
    W = {}
    W["wx_if"] = stack19(Wih0[gi].T, gi, maskcol_if)
    W["wx_go"] = stack19(Wih0[gg].T, gg, zero)
    # decode l0 stream rows: 0:8 = elem (cols 0:8 of Wih0, device-written at
    # partition base 0), 8:16 = ctx (cols 8:16)
    dxi = Wih0[gi].T
    dxg = Wih0[gg].T
    W["dx_if"] = stack19(dxi, gi, maskcol_if)
    W["dx_go"] = stack19(dxg, gg, zero)
    W["w0h_if"] = Whh0[gi].T.copy()   # [64,128]
    W["w0h_go"] = Whh0[gg].T.copy()
    W["w1_if"] = np.concatenate([Wih1[gi].T, Whh1[gi].T], 0)  # [128,128]
    W["w1_go"] = np.concatenate([Wih1[gg].T, Whh1[gg].T], 0)
    W["wdT"] = Wd.T.copy()  # [64,8]
    for k in W:
        W[k] = np.ascontiguousarray(W[k], NPDT)
    W["bd"] = np.ascontiguousarray(bd.reshape(8, 1), np.float32)
    return W


def build_streams(x, lengths_x, context, lengths_ctx):
    """Per-core streams. x [B,TW,16], context [B,TC,8].
    Slot columns: [c0: l0(32) l1(32) | c1: l0(32) l1(32)] where chain c =
    samples [c*32:(c+1)*32] of this core's batch."""
    Bn = x.shape[0]
    TWl = x.shape[1]
    TCl = context.shape[1]

    s_idx = np.arange(SW)
    mw = (s_idx[:, None] < lengths_x[None, :]).astype(np.float32)  # [SW,Bn]
    mw1 = np.zeros_like(mw)
    mw1[1:] = mw[:-1]

    # [CW, 19, 8, chain(2), layer(2), Bc]
    WA = np.zeros((CW, 19, 8, 2, 2, Bc), np.float32)
    xt = np.transpose(x, (1, 2, 0))  # [TW,16,Bn]
    xp = np.concatenate([xt, np.zeros((SW - TWl, 16, Bn), np.float32)], 0)
    WA[:, 0:16, :, :, 0, :] = xp.reshape(CW, 8, 16, 2, Bc).transpose(0, 2, 1, 3, 4)
    WA[:, 16, :, :, 0, :] = (1.0 - mw).reshape(CW, 8, 2, Bc)
    WA[:, 16, :, :, 1, :] = (1.0 - mw1).reshape(CW, 8, 2, Bc)
    WA[:, 17, :, :, 0, :] = 1.0
    WA[:, 18, :, :, 1, :] = 1.0

    # h-capture masks [CW, 128, 8, Bn]: rows 0:64 = mw (h0), 64:128 = mw1 (h1)
    NMw = np.zeros((CW, 128, 8, Bn), np.uint8)
    NMw[:, 0:64] = mw.reshape(CW, 8, 1, Bn).transpose(0, 2, 1, 3)
    NMw[:, 64:128] = mw1.reshape(CW, 8, 1, Bn).transpose(0, 2, 1, 3)

    md0 = np.zeros((SD, Bn), np.float32)
    md0[0:TCl - 1] = 1.0     # l0 steps 0..510 active; 511 pad frozen
    md1 = np.ones((SD, Bn), np.float32)
    md1[0] = 0.0             # freeze l1 at slot 0
    DA = np.zeros((CD, 19, 8, 2, 2, Bc), np.float32)
    ctxt = np.transpose(context, (1, 2, 0))  # [TC,8,Bn]
    cp = np.concatenate(
        [ctxt[0:TCl - 1], np.zeros((SD - (TCl - 1), 8, Bn), np.float32)], 0
    )
    DA[:, 8:16, :, :, 0, :] = cp.reshape(CD, 8, 8, 2, Bc).transpose(0, 2, 1, 3, 4)
    # rows 0:8 (elem) stay 0 in HBM; device fills SBUF copies
    DA[:, 16, :, :, 0, :] = (1.0 - md0).reshape(CD, 8, 2, Bc)
    DA[:, 16, :, :, 1, :] = (1.0 - md1).reshape(CD, 8, 2, Bc)
    DA[:, 17, :, :, 0, :] = 1.0
    DA[:, 18, :, :, 1, :] = 1.0

    def pad1(a):
        return np.concatenate([a, np.zeros_like(a[:1])], 0)
    return dict(
        wa=pad1(WA.reshape(CW, 19, 1024)).astype(NPDT),
        nmw=pad1(NMw.reshape(CW, 128, 512)),
        da=pad1(DA.reshape(CD, 19, 1024)).astype(NPDT),
    )


def post_outputs(YE, YD, bd, lengths_ctx, TCl):
    """YE [8,Bn] fp16, YD [CD,8,8,Bn] f32 -> out [Bn,TCl,8] with -999 pad.
    Chain split uses contiguous sample halves so no reordering is needed."""
    Bn = YE.shape[1]
    out = np.zeros((Bn, TCl, F), np.float32)
    out[:, 0, :] = YE.T.astype(np.float32)
    ysd = YD.transpose(0, 2, 1, 3).reshape(SD, F, Bn)  # [slot, F, Bn]
    # ys_t = slot t+1 for t = 0..510
    out[:, 1:TCl, :] = ysd[1:TCl].transpose(2, 0, 1) + bd[None, None, :]
    valid = np.arange(TCl)[None, :] < lengths_ctx[:, None]
    return np.where(valid[:, :, None], out, np.float32(-999.0))


def build_nc(repeat=1, static=False):
    nc = bass.Bass("TRN2", target_bir_lowering=False, debug=False)

    d = {}
    d["wa"] = nc.dram_tensor("wa", [CW + 1, 19, 1024], MMDT, kind="ExternalInput")
    d["nmw"] = nc.dram_tensor("nmw", [CW + 1, 128, 512], U8, kind="ExternalInput")
    d["da"] = nc.dram_tensor("da", [CD + 1, 19, 1024], MMDT, kind="ExternalInput")
    for name, shp, dt_ in [
        ("wx_if", [19, 128], MMDT), ("wx_go", [19, 128], MMDT),
        ("dx_if", [19, 128], MMDT), ("dx_go", [19, 128], MMDT),
        ("w0h_if", [64, 128], MMDT), ("w0h_go", [64, 128], MMDT),
        ("w1_if", [128, 128], MMDT), ("w1_go", [128, 128], MMDT),
        ("wdT", [64, 8], MMDT), ("bd", [8, 1], F32),
    ]:
        d[name] = nc.dram_tensor(name, shp, dt_, kind="ExternalInput")
    ye = nc.dram_tensor("ye", [8, B], MMDT, kind="ExternalOutput")
    yd = nc.dram_tensor("yd", [CD, 8, 512], F32, kind="ExternalOutput")

    with tile.TileContext(nc) as tc:
        with (
            tc.tile_pool(name="consts", bufs=1) as consts,
            tc.tile_pool(name="state", bufs=1) as state,
            tc.tile_pool(name="stream", bufs=1) as stream,
            tc.tile_pool(name="work", bufs=3) as work,
            tc.tile_pool(name="psum", bufs=2, space="PSUM") as psum,
            tc.tile_pool(name="outp", bufs=1, space="PSUM") as outp,
        ):
            W = {}
            for name in ["wx_if", "wx_go", "dx_if", "dx_go", "w0h_if",
                         "w0h_go", "w1_if", "w1_go"]:
                t = consts.tile(list(d[name].shape), MMDT, tag=name, name="w_" + name)
                nc.sync.dma_start(out=t, in_=d[name][:, :])
                W[name] = t
            wdT_t = consts.tile([128, 8], MMDT, tag="wdT", name="w_wdT")
            nc.sync.dma_start(out=wdT_t[64:128, :], in_=d["wdT"][:, :])
            W["wdT"] = wdT_t
            bd_t = consts.tile([8, 1], F32, tag="bd", name="w_bd")
            nc.sync.dma_start(out=bd_t, in_=d["bd"][:, :])
            W["bd"] = bd_t

            rb = [[state.tile([128, Bc], MMDT, tag=f"rb{c}{i}", name=f"rb{c}{i}")
                   for i in range(2)] for c in range(2)]
            tcc = [[state.tile([128, 2 * Bc], F32, tag=f"tcc{c}{i}", name=f"tcc{c}{i}")
                    for i in range(2)] for c in range(2)]
            for c in range(2):
                for i in range(2):
                    nc.vector.memset(rb[c][i], 0.0)
                    nc.vector.memset(tcc[c][i], 0.0)

            # stream tiles: 8 slots x 128 cols + 128 junk cols (lead matmuls
            # of chain 1 read 64 cols past their slot)
            saA = stream.tile([19, 1152], MMDT, tag="saA")
            saB = stream.tile([19, 1152], MMDT, tag="saB")
            nc.vector.memset(saA[:, 1024:1152], 0.0)
            nc.vector.memset(saB[:, 1024:1152], 0.0)
            nmA = stream.tile([128, 512], U8, tag="nmA")
            nmB = stream.tile([128, 512], U8, tag="nmB")
            elem = state.tile([8, B], MMDT, tag="elem")
            hkeep = state.tile([128, B], MMDT, tag="hkeep")
            nc.vector.memset(hkeep, 0.0)

            def tick_chain(sl, ch, sa, nm, decode, outmm=None):
                par = sl % 2
                rbp, rbn = rb[ch][par], rb[ch][1 - par]
                tccp, tccn = tcc[ch][par], tcc[ch][1 - par]
                t8 = sl % 8
                base = t8 * 128 + ch * 64
                # [IF(64) | GO(64) | junk(64)]; both leads write the junk
                # block so their order (group opener first) is preserved
                mega = psum.tile([128, 3, 64], F32, tag=f"mega{ch}", name=f"mega{ch}")

                wx_if = W["dx_if"] if decode else W["wx_if"]
                wx_go = W["dx_go"] if decode else W["wx_go"]

                nc.tensor.matmul(mega[:, 0::2, :], wx_if, sa[0:19, base:base + 128],
                                 start=True, stop=False)
                nc.tensor.matmul(mega[:, 1:3, :], wx_go, sa[0:19, base:base + 128],
                                 start=False, stop=False)
                nc.tensor.matmul(mega[:, 0, 0:Bc], W["w0h_if"], rbp[0:64, :],
                                 start=False, stop=False)
                nc.tensor.matmul(mega[:, 0, Bc:2 * Bc], W["w1_if"], rbp[:, :],
                                 start=False, stop=False)
                nc.tensor.matmul(mega[:, 1, 0:Bc], W["w0h_go"], rbp[0:64, :],
                                 start=False, stop=False)
                nc.tensor.matmul(mega[:, 1, Bc:2 * Bc], W["w1_go"], rbp[:, :],
                                 start=False, stop=True)
                if outmm is not None:
                    # dense(h1) of the PREVIOUS slot (rbp == rbn of slot-1):
                    # same data dependency as the gate matmuls above
                    ops, oc = outmm
                    nc.tensor.matmul(ops[:, oc * B + ch * Bc:oc * B + (ch + 1) * Bc],
                                     W["wdT"][64:128, :], rbp[64:128, :],
                                     start=True, stop=True)

                # sif = [sigmoid(i); sigmoid(f)] aligned with tcc = [tanh(g); c]
                # so every gate product has equal SBUF base partitions
                sif = work.tile([128, 2 * Bc], F32, tag=f"sif{ch}", name=f"sif{ch}")
                so = work.tile([64, 2 * Bc], F32, tag=f"so{ch}", name=f"so{ch}")
                nc.scalar.activation(sif, mega[:, 0, :], AF.Sigmoid)
                nc.scalar.activation(tccp[0:64, :], mega[0:64, 1, :], AF.Tanh)
                nc.scalar.activation(so, mega[64:128, 1, :], AF.Sigmoid)

                # c' = f*c + i*tanh(g); t2 first (needs only sif, not tg)
                t1 = work.tile([64, 2 * Bc], F32, tag=f"t1{ch}", name=f"t1{ch}")
                t2 = work.tile([64, 2 * Bc], F32, tag=f"t2{ch}", name=f"t2{ch}")
                th = work.tile([64, 2 * Bc], F32, tag=f"th{ch}", name=f"th{ch}")
                nc.vector.tensor_mul(t2, sif[64:128, :], tccp[64:128, :])
                nc.vector.tensor_mul(t1, sif[0:64, :], tccp[0:64, :])
                nc.vector.tensor_add(tccn[64:128, :], t1, t2)
                nc.scalar.activation(th, tccn[64:128, :], AF.Tanh)
                nc.vector.tensor_mul(rbn[0:64, :], so[:, 0:Bc], th[:, 0:Bc])
                nc.vector.tensor_mul(rbn[64:128, :], so[:, Bc:2 * Bc], th[:, Bc:2 * Bc])

                if nm is not None:
                    # capture h at each sample's last active slot
                    mc = t8 * 64 + ch * Bc
                    nc.vector.copy_predicated(hkeep[:, ch * Bc:(ch + 1) * Bc],
                                              nm[:, mc:mc + Bc], rbn)
                return rbn

            def tick(sl, sa, nm, decode, ops=None, oc=None):
                outmm = (ops, oc) if ops is not None else None
                r0 = tick_chain(sl, 0, sa, nm, decode, outmm)
                r1 = tick_chain(sl, 1, sa, nm, decode, outmm)
                return r0, r1

            rep_cm = tc.For_i(0, repeat, 1) if repeat > 1 else contextlib.nullcontext()
            with rep_cm:
                # ================= warmup =================
                nc.sync.dma_start(out=saA[:, 0:1024], in_=d["wa"][0, :, :])
                nc.sync.dma_start(out=nmA, in_=d["nmw"][0, :, :])
                def warm_body(j, i1, i2, first=False):
                    nc.sync.dma_start(out=saB[:, 0:1024], in_=d["wa"][i1, :, :])
                    nc.sync.dma_start(out=nmB, in_=d["nmw"][i1, :, :])
                    for sl in range(8):
                        tick(sl, saA, nmA, False)
                        if first and sl == 0:
                            for c in range(2):
                                nc.vector.memset(rb[c][1][64:128, :], 0.0)
                    nc.sync.dma_start(out=saA[:, 0:1024], in_=d["wa"][i2, :, :])
                    nc.sync.dma_start(out=nmA, in_=d["nmw"][i2, :, :])
                    for sl in range(8, 16):
                        tick(sl, saB, nmB, False)

                if static:
                    for j in range(CW // 2):
                        warm_body(j, j * 2 + 1, j * 2 + 2, first=(j == 0))
                else:
                    warm_body(0, 1, 2, first=True)
                    with tc.For_i(1, CW // 2, 1, hint_engines=(mybir.EngineType.PE,)) as j:
                        warm_body(j, nc.snap(j * 2 + 1), nc.snap(j * 2 + 2))

                # ================= elem =================
                pe = outp.tile([8, B], F32, tag="pe", name="pe")
                for c in range(2):
                    nc.vector.tensor_copy(rb[c][0], hkeep[:, c * Bc:(c + 1) * Bc])
                    nc.tensor.matmul(pe[:, c * Bc:(c + 1) * Bc], W["wdT"][64:128, :],
                                     rb[c][0][64:128, :], start=True, stop=True)
                nc.scalar.activation(elem, pe, AF.Identity, bias=W["bd"][:, 0:1])
                nc.sync.dma_start(out=ye[:, :], in_=elem)
                # broadcast elem into decode stream rows 0:8 (constant input)
                for buf in (saA, saB):
                    for k in range(8):
                        for c in range(2):
                            fb = k * 128 + c * 64
                            nc.vector.tensor_copy(buf[0:8, fb:fb + Bc],
                                                  elem[:, c * Bc:(c + 1) * Bc])
                            nc.vector.memset(buf[0:8, fb + Bc:fb + 64], 0.0)

                # ================= decode =================
                # DMA skips rows 0:8 so the device-written elem rows persist
                def dec_dma(buf, i):
                    nc.sync.dma_start(out=buf[8:19, 0:1024], in_=d["da"][i, 8:19, :])
                dec_dma(saA, 0)
                def dec_body(j, i0, i1, i2, first=False):
                    # tick k emits dense(h1) for slot k-1 (reading rbp); the
                    # last slot's output is emitted in the tail
                    dec_dma(saB, i1)
                    ops = outp.tile([8, 512], F32, tag="ops", name="ops")
                    for sl in range(8):
                        if sl >= 1:
                            tick(sl, saA, None, True, ops, sl - 1)
                        else:
                            tick(sl, saA, None, True)
                        if first and sl == 0:
                            for c in range(2):
                                nc.vector.tensor_copy(rb[c][1][64:128, :],
                                                      rb[c][0][64:128, :])
                    dec_dma(saA, i2)
                    ops2 = outp.tile([8, 512], F32, tag="ops2", name="ops2")
                    rs = None
                    for sl in range(8, 16):
                        if sl == 8:
                            rs = tick(sl, saB, None, True, ops, 7)
                            # PSUM can't be DMA'd directly; the ACT copy here
                            # lands in a queue gap behind this tick's acts
                            oso = work.tile([8, 512], F32, tag="oso", name="oso")
                            nc.scalar.copy(oso, ops)
                            nc.sync.dma_start(out=yd[i0, :, :], in_=oso)
                        else:
                            rs = tick(sl, saB, None, True, ops2, sl - 9)
                    for c in range(2):
                        nc.tensor.matmul(ops2[:, 7 * B + c * Bc:7 * B + (c + 1) * Bc],
                                         W["wdT"][64:128, :], rs[c][64:128, :],
                                         start=True, stop=True)
                    oso2 = work.tile([8, 512], F32, tag="oso2", name="oso2")
                    nc.scalar.copy(oso2, ops2)
                    nc.sync.dma_start(out=yd[i1, :, :], in_=oso2)

                if static:
                    for j in range(CD // 2):
                        dec_body(j, j * 2, j * 2 + 1, j * 2 + 2, first=(j == 0))
                else:
                    dec_body(0, 0, 1, 2, first=True)
                    with tc.For_i(1, CD // 2, 1, hint_engines=(mybir.EngineType.PE,)) as j:
                        dec_body(j, nc.snap(j * 2), nc.snap(j * 2 + 1), nc.snap(j * 2 + 2))

    return nc


def legalize_waits(nc, max_waits=1):
    """walrus codegen caps semaphore waits per instruction; move extras onto
    NoOp instructions inserted immediately before (same engine)."""
    j = json.loads(mybir.module_to_json_bytes(nc.m))
    for fn in j.get("functions", []):
        for blk in fn.get("blocks", []):
            out = []
            for inst in blk.get("instructions", []):
                si = inst.get("sync_info") or {}
                waits = si.get("on_wait") or []
                if len(waits) > max_waits:
                    keep, extra = waits[-max_waits:], waits[:-max_waits]
                    for k, w in enumerate(extra):
                        out.append({"name": f"{inst['name']}-wsp{k}",
                                    "opcode": "NoOp", "engine": inst["engine"],
                                    "ins": [], "outs": [],
                                    "sync_info": {"on_wait": [w], "on_update": []}})
                    si = dict(si); si["on_wait"] = keep
                    inst = dict(inst); inst["sync_info"] = si
                out.append(inst)
            blk["instructions"] = out
    nc.m = mybir.module_from_json_bytes(json.dumps(j).encode())
    return nc


_NC_CACHE = {}


def _get_nc(repeat=1):
    if repeat not in _NC_CACHE:
        nc = build_nc(repeat)
        legalize_waits(nc)
        _NC_CACHE[repeat] = nc
    return _NC_CACHE[repeat]


def build_in_maps(x, lengths_x, context, lengths_ctx,
                  Wih0, Whh0, bih0, bhh0, Wih1, Whh1, bih1, bhh1, Wd, bd):
    Wt = build_weights(np.asarray(Wih0, np.float32), np.asarray(Whh0, np.float32),
                       np.asarray(bih0, np.float32), np.asarray(bhh0, np.float32),
                       np.asarray(Wih1, np.float32), np.asarray(Whh1, np.float32),
                       np.asarray(bih1, np.float32), np.asarray(bhh1, np.float32),
                       np.asarray(Wd, np.float32), np.asarray(bd, np.float32))
    Bn = x.shape[0] // N_CORES
    in_maps = []
    for core in range(N_CORES):
        sl = slice(core * Bn, (core + 1) * Bn)
        st = build_streams(np.ascontiguousarray(x[sl], dtype=np.float32),
                           np.asarray(lengths_x[sl], dtype=np.int64),
                           np.ascontiguousarray(context[sl], dtype=np.float32),
                           np.asarray(lengths_ctx[sl], dtype=np.int64))
        m = dict(st)
        m.update(Wt)
        in_maps.append(m)
    return in_maps


def kernel(x, lengths_x, context, lengths_ctx,
           Wih0, Whh0, bih0, bhh0, Wih1, Whh1, bih1, bhh1, Wd, bd):
    x = np.asarray(x)
    context = np.asarray(context)
    lengths_x = np.asarray(lengths_x)
    lengths_ctx = np.asarray(lengths_ctx)
    in_maps = build_in_maps(x, lengths_x, context, lengths_ctx,
                            np.asarray(Wih0), np.asarray(Whh0), np.asarray(bih0),
                            np.asarray(bhh0), np.asarray(Wih1), np.asarray(Whh1),
                            np.asarray(bih1), np.asarray(bhh1), np.asarray(Wd),
                            np.asarray(bd))
    nc = _get_nc(1)
    res = run_bass_kernel_spmd(nc, in_maps, core_ids=list(range(N_CORES)))
    Bn = x.shape[0] // N_CORES
    outs = []
    bd32 = np.asarray(bd, dtype=np.float32)
    for core in range(N_CORES):
        sl = slice(core * Bn, (core + 1) * Bn)
        YE = res.results[core]["ye"]
        YD = res.results[core]["yd"].reshape(CD, 8, 8, Bn)
        outs.append(post_outputs(YE, YD, bd32,
                                 np.asarray(lengths_ctx[sl], dtype=np.int64), TC))
    return np.concatenate(outs, axis=0).astype(np.float32)



# revision 8
# speedup vs baseline: 2.6136x; 1.1025x over previous
"""Self-contained Trainium2 Bass kernel for nn_AutoRegressive_88837103551116.

2-layer LSTM (HID=64) over ragged sequences: warmup pass over x (per-sample
lengths), then autoregressive decode over [dense(h_top_final), context_t].
Pure data-parallel over 8 NeuronCores (batch 512 -> 64 per core).

Device algorithm (per core):
  - the 64-sample batch is split into TWO independent 32-sample chains that
    interleave on the engines: each chain's recurrence latency hides the
    other's, doubling timestep throughput
  - slot s computes layer0 @ step s and layer1 @ step s-1 (layer stagger)
  - states [feature, batch]: rb [128,Bc]=[h0;h1] (fp16); tcc [128,2Bc] with
    partitions 0:64 = tanh(g) scratch and 64:128 = c state (fp32), so
    i*tanh(g) and f*c merge into one DVE multiply
  - all matmuls fp16 (1 cycle/row on PE); gates accumulate in fp32 PSUM;
    per chain+parity ONE PSUM bank [128,3,64] = [IF | GO | junk]: the junk
    block overlaps both leading matmuls' writes, forcing their order so the
    bank's accumulation group is opened exactly once
  - ONE unified 19-row input stream per slot carries x/ctx (rows 0:16), the
    ragged-sequence c-freeze mask row (+/-BIG into i/f preactivations past
    each sample's length) and per-layer bias rows for both layer column
    halves -> a single leading matmul per gate block; only the two
    h-dependent matmuls per block sit on the recurrence critical path
  - decode folds the constant warmup element into stream rows 8:16 (device
    broadcasts elem into the stream tiles once, DMA skips those rows)
  - h at the last valid warmup step is captured into hkeep via
    copy_predicated with uint8 mask streams (off the critical path)
  - host side: input transposes/stream building, output -999 masking
"""
import sys

import numpy as np

try:
    import concourse.bass as bass
except ImportError:
    sys.path.insert(0, "/opt/trn_rl_repo")
    import concourse.bass as bass

import contextlib
import json

import concourse.tile as tile
from concourse import mybir
from concourse.bass_utils import run_bass_kernel_spmd

N_CORES = 8
TW = 512
TC = 512

H = 64
IN = 16
F = 8
C = 8
BIG = 50.0

F32 = mybir.dt.float32
F16 = mybir.dt.float16
MMDT = F16          # dtype for matmul weights/streams/h-state
NPDT = "float16"   # matching numpy dtype for host-side builders
U8 = mybir.dt.uint8
AF = mybir.ActivationFunctionType

B = 64     # batch per core
Bc = 32    # batch per chain
SW = 528   # warmup slots (padded; needs >= 513)
SD = 512   # decode slots (l0 steps 0..510 + 1 pad)
CW = SW // 8
CD = SD // 8


def build_weights(Wih0, Whh0, bih0, bhh0, Wih1, Whh1, bih1, bhh1, Wd, bd):
    """Stationary lhsT matrices (shared across cores), fp16."""
    b0 = bih0 + bhh0
    b1 = bih1 + bhh1
    maskcol_if = np.concatenate([np.full(H, -BIG), np.full(H, BIG)]).astype(np.float32)
    zero = np.zeros(128, np.float32)

    def stack19(xw_rows, gate_rows, mask):
        # lhsT [19, 128]: rows 0:16 x-weights, 16 mask, 17 b0, 18 b1
        out = np.zeros((19, 128), np.float32)
        out[0:xw_rows.shape[0], :] = xw_rows
        out[16] = mask
        out[17] = b0[gate_rows]
        out[18] = b1[gate_rows]
        return out

    gi = slice(0, 128)    # i,f rows
    gg = slice(128, 256)  # g,o rows
    W = {}
    W["wx_if"] = stack19(Wih0[gi].T, gi, maskcol_if)
    W["wx_go"] = stack19(Wih0[gg].T, gg, zero)
    # decode l0 stream rows: 0:8 = elem (cols 0:8 of Wih0, device-written at
    # partition base 0), 8:16 = ctx (cols 8:16)
    dxi = Wih0[gi].T
    dxg = Wih0[gg].T
    W["dx_if"] = stack19(dxi, gi, maskcol_if)
    W["dx_go"] = stack19(dxg, gg, zero)
    W["w0h_if"] = Whh0[gi].T.copy()   # [64,128]
    W["w0h_go"] = Whh0[gg].T.copy()
    W["w1_if"] = np.concatenate([Wih1[gi].T, Whh1[gi].T], 0)  # [128,128]
    W["w1_go"] = np.concatenate([Wih1[gg].T, Whh1[gg].T], 0)
    W["wdT"] = Wd.T.copy()  # [64,8]
    for k in W:
        W[k] = np.ascontiguousarray(W[k], NPDT)
    W["bd"] = np.ascontiguousarray(bd.reshape(8, 1), np.float32)
    return W


def build_streams(x, lengths_x, context, lengths_ctx):
    """Per-core streams. x [B,TW,16], context [B,TC,8].
    Slot columns: [c0: l0(32) l1(32) | c1: l0(32) l1(32)] where chain c =
    samples [c*32:(c+1)*32] of this core's batch."""
    Bn = x.shape[0]
    TWl = x.shape[1]
    TCl = context.shape[1]

    s_idx = np.arange(SW)
    mw = (s_idx[:, None] < lengths_x[None, :]).astype(np.float32)  # [SW,Bn]
    mw1 = np.zeros_like(mw)
    mw1[1:] = mw[:-1]

    # [CW, 19, 8, chain(2), layer(2), Bc]
    WA = np.zeros((CW, 19, 8, 2, 2, Bc), np.float32)
    xt = np.transpose(x, (1, 2, 0))  # [TW,16,Bn]
    xp = np.concatenate([xt, np.zeros((SW - TWl, 16, Bn), np.float32)], 0)
    WA[:, 0:16, :, :, 0, :] = xp.reshape(CW, 8, 16, 2, Bc).transpose(0, 2, 1, 3, 4)
    WA[:, 16, :, :, 0, :] = (1.0 - mw).reshape(CW, 8, 2, Bc)
    WA[:, 16, :, :, 1, :] = (1.0 - mw1).reshape(CW, 8, 2, Bc)
    WA[:, 17, :, :, 0, :] = 1.0
    WA[:, 18, :, :, 1, :] = 1.0

    # h-capture masks [CW, 128, 8, Bn]: rows 0:64 = mw (h0), 64:128 = mw1 (h1)
    NMw = np.zeros((CW, 128, 8, Bn), np.uint8)
    NMw[:, 0:64] = mw.reshape(CW, 8, 1, Bn).transpose(0, 2, 1, 3)
    NMw[:, 64:128] = mw1.reshape(CW, 8, 1, Bn).transpose(0, 2, 1, 3)

    md0 = np.zeros((SD, Bn), np.float32)
    md0[0:TCl - 1] = 1.0     # l0 steps 0..510 active; 511 pad frozen
    md1 = np.ones((SD, Bn), np.float32)
    md1[0] = 0.0             # freeze l1 at slot 0
    DA = np.zeros((CD, 19, 8, 2, 2, Bc), np.float32)
    ctxt = np.transpose(context, (1, 2, 0))  # [TC,8,Bn]
    cp = np.concatenate(
        [ctxt[0:TCl - 1], np.zeros((SD - (TCl - 1), 8, Bn), np.float32)], 0
    )
    DA[:, 8:16, :, :, 0, :] = cp.reshape(CD, 8, 8, 2, Bc).transpose(0, 2, 1, 3, 4)
    # rows 0:8 (elem) stay 0 in HBM; device fills SBUF copies
    DA[:, 16, :, :, 0, :] = (1.0 - md0).reshape(CD, 8, 2, Bc)
    DA[:, 16, :, :, 1, :] = (1.0 - md1).reshape(CD, 8, 2, Bc)
    DA[:, 17, :, :, 0, :] = 1.0
    DA[:, 18, :, :, 1, :] = 1.0

    def pad1(a):
        return np.concatenate([a, np.zeros_like(a[:1])], 0)
    return dict(
        wa=pad1(WA.reshape(CW, 19, 1024)).astype(NPDT),
        nmw=pad1(NMw.reshape(CW, 128, 512)),
        da=pad1(DA.reshape(CD, 19, 1024)).astype(NPDT),
    )


def post_outputs(YE, YD, bd, lengths_ctx, TCl):
    """YE [8,Bn] fp16, YD [CD,8,8,Bn] f32 -> out [Bn,TCl,8] with -999 pad.
    Chain split uses contiguous sample halves so no reordering is needed."""
    Bn = YE.shape[1]
    out = np.zeros((Bn, TCl, F), np.float32)
    out[:, 0, :] = YE.T.astype(np.float32)
    ysd = YD.transpose(0, 2, 1, 3).reshape(SD, F, Bn)  # [slot, F, Bn]
    # ys_t = slot t+1 for t = 0..510
    out[:, 1:TCl, :] = ysd[1:TCl].transpose(2, 0, 1) + bd[None, None, :]
    valid = np.arange(TCl)[None, :] < lengths_ctx[:, None]
    return np.where(valid[:, :, None], out, np.float32(-999.0))


def build_nc(repeat=1, static=False):
    nc = bass.Bass("TRN2", target_bir_lowering=False, debug=False)

    d = {}
    d["wa"] = nc.dram_tensor("wa", [CW + 1, 19, 1024], MMDT, kind="ExternalInput")
    d["nmw"] = nc.dram_tensor("nmw", [CW + 1, 128, 512], U8, kind="ExternalInput")
    d["da"] = nc.dram_tensor("da", [CD + 1, 19, 1024], MMDT, kind="ExternalInput")
    for name, shp, dt_ in [
        ("wx_if", [19, 128], MMDT), ("wx_go", [19, 128], MMDT),
        ("dx_if", [19, 128], MMDT), ("dx_go", [19, 128], MMDT),
        ("w0h_if", [64, 128], MMDT), ("w0h_go", [64, 128], MMDT),
        ("w1_if", [128, 128], MMDT), ("w1_go", [128, 128], MMDT),
        ("wdT", [64, 8], MMDT), ("bd", [8, 1], F32),
    ]:
        d[name] = nc.dram_tensor(name, shp, dt_, kind="ExternalInput")
    ye = nc.dram_tensor("ye", [8, B], MMDT, kind="ExternalOutput")
    yd = nc.dram_tensor("yd", [CD, 8, 512], F32, kind="ExternalOutput")

    with tile.TileContext(nc) as tc:
        with (
            tc.tile_pool(name="consts", bufs=1) as consts,
            tc.tile_pool(name="state", bufs=1) as state,
            tc.tile_pool(name="stream", bufs=1) as stream,
            tc.tile_pool(name="work", bufs=3) as work,
            tc.tile_pool(name="psum", bufs=1, space="PSUM") as psum,
            tc.tile_pool(name="outp", bufs=1, space="PSUM") as outp,
        ):
            W = {}
            for name in ["wx_if", "wx_go", "dx_if", "dx_go", "w0h_if",
                         "w0h_go", "w1_if", "w1_go"]:
                t = consts.tile(list(d[name].shape), MMDT, tag=name, name="w_" + name)
                nc.sync.dma_start(out=t, in_=d[name][:, :])
                W[name] = t
            wdT_t = consts.tile([128, 8], MMDT, tag="wdT", name="w_wdT")
            nc.sync.dma_start(out=wdT_t[64:128, :], in_=d["wdT"][:, :])
            W["wdT"] = wdT_t
            bd_t = consts.tile([8, 1], F32, tag="bd", name="w_bd")
            nc.sync.dma_start(out=bd_t, in_=d["bd"][:, :])
            W["bd"] = bd_t

            rb = [[state.tile([128, Bc], MMDT, tag=f"rb{c}{i}", name=f"rb{c}{i}")
                   for i in range(2)] for c in range(2)]
            tcc = [[state.tile([128, 2 * Bc], F32, tag=f"tcc{c}{i}", name=f"tcc{c}{i}")
                    for i in range(2)] for c in range(2)]
            for c in range(2):
                for i in range(2):
                    nc.vector.memset(rb[c][i], 0.0)
                    nc.vector.memset(tcc[c][i], 0.0)

            # stream tiles: 8 slots x 128 cols
            saA = stream.tile([19, 1024], MMDT, tag="saA")
            saB = stream.tile([19, 1024], MMDT, tag="saB")
            nmA = stream.tile([128, 512], U8, tag="nmA")
            nmB = stream.tile([128, 512], U8, tag="nmB")
            elem = state.tile([8, B], MMDT, tag="elem")
            hkeep = state.tile([128, B], MMDT, tag="hkeep")
            nc.vector.memset(hkeep, 0.0)

            def tick(sl, sa, nm, decode, ops=None, oc=None):
                """Phase-interleaved emission across both chains so neither
                engine FIFO stalls on the other chain's pending data."""
                par = sl % 2
                t8 = sl % 8
                rbp = [rb[c][par] for c in range(2)]
                rbn = [rb[c][1 - par] for c in range(2)]
                tccp = [tcc[c][par] for c in range(2)]
                tccn = [tcc[c][1 - par] for c in range(2)]
                wx_if = W["dx_if"] if decode else W["wx_if"]
                wx_go = W["dx_go"] if decode else W["wx_go"]
                megi, mego, sif, so, t1, t2, th = ({} for _ in range(7))

                for ch in range(2):
                    base = t8 * 128 + ch * 64
                    # separate PSUM banks per gate block; each lead opens its
                    # own bank's accumulation group by writing the full tile
                    megi[ch] = psum.tile([128, 2 * Bc], F32, tag=f"mi{ch}",
                                         name=f"mi{ch}")
                    mego[ch] = psum.tile([128, 2 * Bc], F32, tag=f"mo{ch}",
                                         name=f"mo{ch}")
                    nc.tensor.matmul(mego[ch], wx_go, sa[0:19, base:base + 64],
                                     start=True, stop=False)
                    nc.tensor.matmul(megi[ch], wx_if, sa[0:19, base:base + 64],
                                     start=True, stop=False)
                    # GO h-matmuls first: tanh(g) unblocks two matmuls earlier
                    nc.tensor.matmul(mego[ch][:, 0:Bc], W["w0h_go"],
                                     rbp[ch][0:64, :], start=False, stop=False)
                    nc.tensor.matmul(mego[ch][:, Bc:2 * Bc], W["w1_go"],
                                     rbp[ch][:, :], start=False, stop=True)
                    nc.tensor.matmul(megi[ch][:, 0:Bc], W["w0h_if"],
                                     rbp[ch][0:64, :], start=False, stop=False)
                    nc.tensor.matmul(megi[ch][:, Bc:2 * Bc], W["w1_if"],
                                     rbp[ch][:, :], start=False, stop=True)
                    if ops is not None:
                        # dense(h1) of the PREVIOUS slot (rbp == rbn of
                        # slot-1): same data dependency as the gate matmuls
                        nc.tensor.matmul(
                            ops[:, oc * B + ch * Bc:oc * B + (ch + 1) * Bc],
                            W["wdT"][64:128, :], rbp[ch][64:128, :],
                            start=True, stop=True)

                for ch in range(2):
                    # sif = [sig(i); sig(f)] aligned with tcc = [tanh(g); c]
                    sif[ch] = work.tile([128, 2 * Bc], F32, tag=f"sif{ch}",
                                        name=f"sif{ch}")
                    so[ch] = work.tile([64, 2 * Bc], F32, tag=f"so{ch}",
                                       name=f"so{ch}")
                    nc.scalar.activation(tccp[ch][0:64, :], mego[ch][0:64, :],
                                         AF.Tanh)
                    nc.scalar.activation(so[ch], mego[ch][64:128, :], AF.Sigmoid)
                    nc.scalar.activation(sif[ch], megi[ch], AF.Sigmoid)

                for ch in range(2):
                    # c' = f*c + i*tanh(g); t2 first (needs only sif, not tg)
                    t1[ch] = work.tile([64, 2 * Bc], F32, tag=f"t1{ch}",
                                       name=f"t1{ch}")
                    t2[ch] = work.tile([64, 2 * Bc], F32, tag=f"t2{ch}",
                                       name=f"t2{ch}")
                    nc.vector.tensor_mul(t2[ch], sif[ch][64:128, :],
                                         tccp[ch][64:128, :])
                    nc.vector.tensor_mul(t1[ch], sif[ch][0:64, :],
                                         tccp[ch][0:64, :])
                    nc.vector.tensor_add(tccn[ch][64:128, :], t1[ch], t2[ch])

                for ch in range(2):
                    th[ch] = work.tile([64, 2 * Bc], F32, tag=f"th{ch}",
                                       name=f"th{ch}")
                    nc.scalar.activation(th[ch], tccn[ch][64:128, :], AF.Tanh)

                for ch in range(2):
                    nc.vector.tensor_mul(rbn[ch][0:64, :], so[ch][:, 0:Bc],
                                         th[ch][:, 0:Bc])
                    nc.vector.tensor_mul(rbn[ch][64:128, :], so[ch][:, Bc:2 * Bc],
                                         th[ch][:, Bc:2 * Bc])

                if nm is not None:
                    for ch in range(2):
                        # capture h at each sample's last active slot
                        mc = t8 * 64 + ch * Bc
                        nc.vector.copy_predicated(hkeep[:, ch * Bc:(ch + 1) * Bc],
                                                  nm[:, mc:mc + Bc], rbn[ch])
                return rbn[0], rbn[1]

            rep_cm = tc.For_i(0, repeat, 1) if repeat > 1 else contextlib.nullcontext()
            with rep_cm:
                # ================= warmup =================
                nc.sync.dma_start(out=saA[:, 0:1024], in_=d["wa"][0, :, :])
                nc.sync.dma_start(out=nmA, in_=d["nmw"][0, :, :])
                def warm_body(j, i1, i2, first=False):
                    nc.sync.dma_start(out=saB[:, 0:1024], in_=d["wa"][i1, :, :])
                    nc.sync.dma_start(out=nmB, in_=d["nmw"][i1, :, :])
                    for sl in range(8):
                        tick(sl, saA, nmA, False)
                        if first and sl == 0:
                            for c in range(2):
                                nc.vector.memset(rb[c][1][64:128, :], 0.0)
                    nc.sync.dma_start(out=saA[:, 0:1024], in_=d["wa"][i2, :, :])
                    nc.sync.dma_start(out=nmA, in_=d["nmw"][i2, :, :])
                    for sl in range(8, 16):
                        tick(sl, saB, nmB, False)

                if static:
                    for j in range(CW // 2):
                        warm_body(j, j * 2 + 1, j * 2 + 2, first=(j == 0))
                else:
                    warm_body(0, 1, 2, first=True)
                    with tc.For_i(1, CW // 2, 1, hint_engines=(mybir.EngineType.PE,)) as j:
                        warm_body(j, nc.snap(j * 2 + 1), nc.snap(j * 2 + 2))

                # ================= elem =================
                pe = outp.tile([8, B], F32, tag="pe", name="pe")
                for c in range(2):
                    nc.vector.tensor_copy(rb[c][0], hkeep[:, c * Bc:(c + 1) * Bc])
                    nc.tensor.matmul(pe[:, c * Bc:(c + 1) * Bc], W["wdT"][64:128, :],
                                     rb[c][0][64:128, :], start=True, stop=True)
                nc.scalar.activation(elem, pe, AF.Identity, bias=W["bd"][:, 0:1])
                nc.sync.dma_start(out=ye[:, :], in_=elem)
                # broadcast elem into decode stream rows 0:8 (constant input)
                for buf in (saA, saB):
                    for k in range(8):
                        for c in range(2):
                            fb = k * 128 + c * 64
                            nc.vector.tensor_copy(buf[0:8, fb:fb + Bc],
                                                  elem[:, c * Bc:(c + 1) * Bc])
                            nc.vector.memset(buf[0:8, fb + Bc:fb + 64], 0.0)

                # ================= decode =================
                # DMA skips rows 0:8 so the device-written elem rows persist
                def dec_dma(buf, i):
                    nc.sync.dma_start(out=buf[8:19, 0:1024], in_=d["da"][i, 8:19, :])
                dec_dma(saA, 0)
                def dec_body(j, i0, i1, i2, first=False):
                    # tick k emits dense(h1) for slot k-1 (reading rbp); the
                    # last slot's output is emitted in the tail
                    dec_dma(saB, i1)
                    ops = outp.tile([8, 512], F32, tag="ops", name="ops")
                    for sl in range(8):
                        if sl >= 1:
                            tick(sl, saA, None, True, ops, sl - 1)
                        else:
                            tick(sl, saA, None, True)
                        if first and sl == 0:
                            for c in range(2):
                                nc.vector.tensor_copy(rb[c][1][64:128, :],
                                                      rb[c][0][64:128, :])
                    dec_dma(saA, i2)
                    ops2 = outp.tile([8, 512], F32, tag="ops2", name="ops2")
                    rs = None
                    for sl in range(8, 16):
                        if sl == 8:
                            rs = tick(sl, saB, None, True, ops, 7)
                            # PSUM can't be DMA'd directly; the ACT copy here
                            # lands in a queue gap behind this tick's acts
                            oso = work.tile([8, 512], F32, tag="oso", name="oso")
                            nc.scalar.copy(oso, ops)
                            nc.sync.dma_start(out=yd[i0, :, :], in_=oso)
                        else:
                            rs = tick(sl, saB, None, True, ops2, sl - 9)
                    for c in range(2):
                        nc.tensor.matmul(ops2[:, 7 * B + c * Bc:7 * B + (c + 1) * Bc],
                                         W["wdT"][64:128, :], rs[c][64:128, :],
                                         start=True, stop=True)
                    oso2 = work.tile([8, 512], F32, tag="oso2", name="oso2")
                    nc.scalar.copy(oso2, ops2)
                    nc.sync.dma_start(out=yd[i1, :, :], in_=oso2)

                if static:
                    for j in range(CD // 2):
                        dec_body(j, j * 2, j * 2 + 1, j * 2 + 2, first=(j == 0))
                else:
                    dec_body(0, 0, 1, 2, first=True)
                    with tc.For_i(1, CD // 2, 1, hint_engines=(mybir.EngineType.PE,)) as j:
                        dec_body(j, nc.snap(j * 2), nc.snap(j * 2 + 1), nc.snap(j * 2 + 2))

    return nc


def legalize_waits(nc, max_waits=1):
    """walrus codegen caps semaphore waits per instruction; move extras onto
    NoOp instructions inserted immediately before (same engine)."""
    j = json.loads(mybir.module_to_json_bytes(nc.m))
    for fn in j.get("functions", []):
        for blk in fn.get("blocks", []):
            out = []
            for inst in blk.get("instructions", []):
                si = inst.get("sync_info") or {}
                waits = si.get("on_wait") or []
                if len(waits) > max_waits:
                    keep, extra = waits[-max_waits:], waits[:-max_waits]
                    for k, w in enumerate(extra):
                        out.append({"name": f"{inst['name']}-wsp{k}",
                                    "opcode": "NoOp", "engine": inst["engine"],
                                    "ins": [], "outs": [],
                                    "sync_info": {"on_wait": [w], "on_update": []}})
                    si = dict(si); si["on_wait"] = keep
                    inst = dict(inst); inst["sync_info"] = si
                out.append(inst)
            blk["instructions"] = out
    nc.m = mybir.module_from_json_bytes(json.dumps(j).encode())
    return nc


_NC_CACHE = {}


def _get_nc(repeat=1):
    if repeat not in _NC_CACHE:
        nc = build_nc(repeat)
        legalize_waits(nc)
        _NC_CACHE[repeat] = nc
    return _NC_CACHE[repeat]


def build_in_maps(x, lengths_x, context, lengths_ctx,
                  Wih0, Whh0, bih0, bhh0, Wih1, Whh1, bih1, bhh1, Wd, bd):
    Wt = build_weights(np.asarray(Wih0, np.float32), np.asarray(Whh0, np.float32),
                       np.asarray(bih0, np.float32), np.asarray(bhh0, np.float32),
                       np.asarray(Wih1, np.float32), np.asarray(Whh1, np.float32),
                       np.asarray(bih1, np.float32), np.asarray(bhh1, np.float32),
                       np.asarray(Wd, np.float32), np.asarray(bd, np.float32))
    Bn = x.shape[0] // N_CORES
    in_maps = []
    for core in range(N_CORES):
        sl = slice(core * Bn, (core + 1) * Bn)
        st = build_streams(np.ascontiguousarray(x[sl], dtype=np.float32),
                           np.asarray(lengths_x[sl], dtype=np.int64),
                           np.ascontiguousarray(context[sl], dtype=np.float32),
                           np.asarray(lengths_ctx[sl], dtype=np.int64))
        m = dict(st)
        m.update(Wt)
        in_maps.append(m)
    return in_maps


def kernel(x, lengths_x, context, lengths_ctx,
           Wih0, Whh0, bih0, bhh0, Wih1, Whh1, bih1, bhh1, Wd, bd):
    x = np.asarray(x)
    context = np.asarray(context)
    lengths_x = np.asarray(lengths_x)
    lengths_ctx = np.asarray(lengths_ctx)
    in_maps = build_in_maps(x, lengths_x, context, lengths_ctx,
                            np.asarray(Wih0), np.asarray(Whh0), np.asarray(bih0),
                            np.asarray(bhh0), np.asarray(Wih1), np.asarray(Whh1),
                            np.asarray(bih1), np.asarray(bhh1), np.asarray(Wd),
                            np.asarray(bd))
    nc = _get_nc(1)
    res = run_bass_kernel_spmd(nc, in_maps, core_ids=list(range(N_CORES)))
    Bn = x.shape[0] // N_CORES
    outs = []
    bd32 = np.asarray(bd, dtype=np.float32)
    for core in range(N_CORES):
        sl = slice(core * Bn, (core + 1) * Bn)
        YE = res.results[core]["ye"]
        YD = res.results[core]["yd"].reshape(CD, 8, 8, Bn)
        outs.append(post_outputs(YE, YD, bd32,
                                 np.asarray(lengths_ctx[sl], dtype=np.int64), TC))
    return np.concatenate(outs, axis=0).astype(np.float32)

